# revision 35
# baseline (speedup 1.0000x reference)
"""Multi-head causal attention (B=2, S=2048, D=1024, H=16, dh=64) on 8 TRN2 cores.

Strategy (53.7 us baseline -> this version)
-------------------------------------------
- Shard the 32 (batch, head) pairs across 8 cores, 4 pairs each. Two heads
  packed per 128 SBUF partitions (64 dh-rows each). Pure data parallel.
- S^T = K @ Q^T on the PE in [k, q] layout. NEW: chunks qc>=1 (query rows
  512..2047) run in fp8e4 with MatmulPerfMode.DoubleRow (0.5 cyc/col, 2x):
  the two DoubleRow contraction slices carry K_hi and K_lo (hi/lo fp8
  decomposition, K effectively exact); Q is single fp8 (score noise
  sigma~0.036, harmless for rows with many softmax terms). Chunk 0 (rows
  0..511, few-term softmax, noise-sensitive) stays bf16.
- Causal masking via PE bias-matmuls instead of Pool mask-multiplies: a
  DoubleRow fp8 matmul with split-identity lhsT accumulates a -80 bias onto
  the masked entries of each diagonal 128-col strip (PSUM group: S start,
  bias stop). exp(-80-ish) underflows to ~1e-35 on every exp path (the i16
  Schraudolph wraps into small positive int16s -> tiny positive bf16).
- exp(S - 1.5) (global shift, cancelled by the softmax division; keeps all
  paths comfortably in range) is split across THREE engines: ACT (true exp,
  bias=-1.5), DVE and Pool (Schraudolph bit-trick tensor_scalar:
  int16(x*128/ln2 + B') bitcast bf16, ~4% log-quantization). Chunk-0 exp is
  forced to ACT (true exp) since Schraudolph noise hurts short rows.
  A static greedy planner balances per-engine busy time.
- P@V runs SWAPPED all-bf16: P^T [128k x 128q] stationary, V [128k x 65]
  moving (65 cols per k-block, full 128 output partitions). Row sums l via a
  ones-column in V; host divides. (fp8 DoubleRow P@V is impossible: it would
  need P^T folded to 64 partitions, which no engine can do for free.)
- The whole (group, chunk, block) stream is software-pipelined flat: P@V
  retires LA blocks behind S^T/exp. PSUM: 3 score tiles + 2 output banks.
- Input DMAs on the SP HWDGE queue (small leading K/Q slice first, then
  chunk-sized pieces ordered by first use); output stores on the Pool SWDGE
  queue; drain store split across two queues.
"""

import os
import sys
from contextlib import ExitStack

import numpy as np

for _p in ("/opt/trn_rl_repo", "/root/.axon_site/_ro/trn_rl_repo"):
    if os.path.isdir(_p) and _p not in sys.path:
        sys.path.insert(0, _p)
        break

import concourse.bacc as bacc  # noqa: E402
import concourse.mybir as mybir  # noqa: E402
import concourse.tile as tile  # noqa: E402
from concourse.bass_utils import run_bass_kernel_spmd  # noqa: E402

F32 = mybir.dt.float32
BF16 = mybir.dt.bfloat16
FP8 = mybir.dt.float8e4
I16 = mybir.dt.int16
EXP = mybir.ActivationFunctionType.Exp
MULT = mybir.AluOpType.mult
ADD = mybir.AluOpType.add
DR = mybir.MatmulPerfMode.DoubleRow

N_CORES = 8
H = 16
DH = 64
QBLK = 512
KBLK = 128
VW = DH + 1

USE_FP8 = os.environ.get("K_FP8", "1") == "1"
USE_SCHRAUD = os.environ.get("K_SCHRAUD", "1") == "1"
NOBIAS = os.environ.get("K_NOBIAS", "0") == "1"  # debug: skip mask bias

# Schraudolph constants for bf16: i16 = x * (2^7/ln2) + SCH_B, bitcast bf16.
SCH_A = 128.0 / float(np.log(2.0))
SCH_B = 16249.0
CSH = float(os.environ.get("K_CSH", "1.5"))  # global exp shift
MBIAS = -80.0  # additive mask bias (keeps Schraudolph int16 small-positive)

# engine-time model used only for the static work split (ns); overheads
# measured from CoreSim (marginal per-instruction cost)
ACT_RATE, ACT_OVH = 0.833, float(os.environ.get("K_AOVH", "185"))
DVE_RATE, DVE_OVH = 1.042, float(os.environ.get("K_VOVH", "130"))
POOL_RATE, POOL_OVH = float(os.environ.get("K_PRATE", "1.389")), \
    float(os.environ.get("K_POVH", "230"))

LAST_RESULTS = None  # BassKernelResults of the most recent kernel() call


def _plan_blocks(mask):
    """Classify [KBLK x QBLK] blocks of S^T per q-chunk, union over batch.

    Returns (plans, uniq_contents):
      plans[qc] = list of (kk, c0, c1, m0, m1, uid); block covers k rows
        kk*KBLK..+KBLK and q columns qc*QBLK+c0..qc*QBLK+c1. If uid >= 0,
        the strip [m0, m1) needs the mask-bias matmul with tile `uid`.
      uniq_contents[uid] = float32 [B, KBLK, mw] 0/1 tile (per-batch content).
    """
    B, S, _ = mask.shape
    NQ, NK = S // QBLK, S // KBLK
    uniq_keys = {}
    uniq_contents = []
    plans = []
    for qc in range(NQ):
        out = []
        for kk in range(NK):
            sub = mask[:, qc * QBLK:(qc + 1) * QBLK, kk * KBLK:(kk + 1) * KBLK]
            anyk = sub.any(axis=(0, 2))  # [QBLK] column needed?
            if not anyk.any():
                continue
            # swap-mode P@V slices lhsT at 128-aligned q-subblocks
            c0 = int(anyk.argmax()) & ~(KBLK - 1)
            c1 = min(QBLK,
                     (QBLK - int(anyk[::-1].argmax()) + KBLK - 1) & ~(KBLK - 1))
            allk = sub.all(axis=(0, 2))
            dirty = ~allk
            dirty[:c0] = False
            dirty[c1:] = False
            if dirty.any():
                m0 = int(dirty.argmax()) & ~3
                m1 = min(QBLK, (QBLK - int(dirty[::-1].argmax()) + 3) & ~3)
                dirty[m0:m1] = True
                content = np.zeros((B, KBLK, m1 - m0), np.float32)
                for bb in range(B):
                    content[bb] = sub[bb, m0:m1, :].T
                key = content.tobytes()
                uid = uniq_keys.get(key)
                if uid is None:
                    uid = len(uniq_contents)
                    uniq_keys[key] = uid
                    uniq_contents.append(content)
            else:
                m0 = m1 = 0
                uid = -1
            out.append((kk, c0, c1, m0, m1, uid))
        plans.append(out)
    mw = max((c.shape[2] for c in uniq_contents), default=1)
    uniq_padded = []
    for c in uniq_contents:
        p = np.zeros((B, KBLK, mw), np.float32)
        p[:, :, :c.shape[2]] = c
        uniq_padded.append(p)
    return plans, uniq_padded


def _flat_order(plans, n_groups):
    """Emission order: per group, chunk-0 blocks woven between chunk-1's
    (chunk-0 exp is ACT-pinned, weaving keeps both engines fed through the
    3-deep PSUM score ring), then chunks 2, 3."""
    NQ = len(plans)
    out = []
    for gi in range(n_groups):
        if NQ == 4 and USE_SCHRAUD and USE_FP8:
            q0, q1 = plans[0], plans[1]
            i0 = 0
            for j, blk in enumerate(q1):
                out.append((gi, 1, j, blk, len(q1)))
                if j % 2 == 1 and i0 < len(q0):
                    out.append((gi, 0, i0, q0[i0], len(q0)))
                    i0 += 1
            while i0 < len(q0):
                out.append((gi, 0, i0, q0[i0], len(q0)))
                i0 += 1
            for qc in (2, 3):
                for j, blk in enumerate(plans[qc]):
                    out.append((gi, qc, j, blk, len(plans[qc])))
        else:
            for qc in range(NQ):
                for j, blk in enumerate(plans[qc]):
                    out.append((gi, qc, j, blk, len(plans[qc])))
    return out


def _plan_engines(S, n_groups, plans):
    """Ring-aware list scheduler: simulate PE pace, the s_bufs-deep PSUM
    score ring, and ACT/DVE availability; assign each block's exp to the
    engine that finishes it earliest. Chunk-0 exp is pinned to ACT (true
    exp; Schraudolph noise hurts short softmax rows).

    Returns (assign, loads): assign keyed (gi, qc, kk) -> 'a'|'v' plus
    ('copy', gi, qc) -> 'a'|'v'.
    """
    rates = {"a": ACT_RATE, "v": DVE_RATE}
    ovhs = {"a": ACT_OVH, "v": DVE_OVH}
    sb = int(os.environ.get("K_SBUFS", "3"))
    pe_c = 0.41667
    order = _flat_order(plans, n_groups)
    assign = {}
    eng_free = {"a": float(os.environ.get("K_APRE", "0")), "v": 0.0}
    load = {"a": eng_free["a"], "v": 0.0}
    slot_free = [0.0] * sb
    t_pe = 0.0
    for i, (gi, qc, bi, blk, nb) in enumerate(order):
        kk, c0, c1, m0, m1, uid = blk
        W = c1 - c0
        use8 = USE_FP8 and qc >= 1
        s_dur = W * pe_c * (1.0 if use8 else 2.0)
        if uid >= 0:
            s_dur += (m1 - m0) * pe_c * (1.0 if use8 else 2.0)
        pv_dur = (W // KBLK) * 2 * VW * pe_c
        s_start = max(t_pe, slot_free[i % sb])
        s_done = s_start + s_dur
        t_pe = s_done + pv_dur  # an earlier block's P@V paces in between
        cols = 2 * W
        pin = "a" if (qc == 0 and USE_SCHRAUD and USE_FP8) else None
        best, bestend = None, None
        for e in ("a", "v"):
            if not USE_SCHRAUD and e != "a":
                continue
            if pin is not None and e != pin:
                continue
            end = max(s_done, eng_free[e]) + rates[e] * cols + ovhs[e]
            if bestend is None or end < bestend:
                best, bestend = e, end
        assign[(gi, qc, kk)] = best
        eng_free[best] = bestend
        load[best] += rates[best] * cols + ovhs[best]
        slot_free[i % sb] = bestend
        if bi == nb - 1:
            ck = ("copy", gi, qc)
            best, bestend = None, None
            for e in ("a", "v"):
                end = eng_free[e] + rates[e] * 8 * VW + ovhs[e]
                if bestend is None or end < bestend:
                    best, bestend = e, end
            assign[ck] = best
            eng_free[best] = bestend
            load[best] += rates[best] * 8 * VW + ovhs[best]
    return assign, load


def _build(S, n_groups, n_pairs, plans, n_uniq, mw=1,
           la=None, p_bufs=None, s_bufs=None, o_bufs=None, osb_bufs=None):
    if la is None:
        la = int(os.environ.get("K_LA", "5"))
    if p_bufs is None:
        p_bufs = int(os.environ.get("K_PBUFS", "15"))
    if osb_bufs is None:
        osb_bufs = int(os.environ.get("K_OSB", "4"))
    if s_bufs is None:
        s_bufs = int(os.environ.get("K_SBUFS", "3"))
    if o_bufs is None:
        o_bufs = int(os.environ.get("K_OBUFS", "1"))
    """Build the single SPMD program run identically on all cores."""
    NQ, NK = S // QBLK, S // KBLK
    nc = bacc.Bacc("TRN2", target_bir_lowering=False, debug=False)
    # bf16 K/Q: only chunk-0 columns (0..QBLK)
    qtb = nc.declare_dram_parameter("qtb", [n_groups, 128, QBLK], BF16,
                                    isOutput=False)
    ktb = nc.declare_dram_parameter("ktb", [n_groups, 128, QBLK], BF16,
                                    isOutput=False)
    # fp8 K hi/lo on partitions 0..63 (DoubleRow requires base partition 0):
    # [g, p, h*(NK*256) + kk*256 + i*128 + m]
    k8 = nc.declare_dram_parameter("k8", [n_groups, 64, 2 * NK * 2 * KBLK],
                                   FP8, isOutput=False)
    # fp8 Q columns QBLK..S, heads side by side: [g, p, h*(S-QBLK) + q]
    q8 = nc.declare_dram_parameter("q8", [n_groups, 64, 2 * (S - QBLK)], FP8,
                                   isOutput=False)
    vv = nc.declare_dram_parameter("vv", [n_pairs, 128, NK * VW], BF16,
                                   isOutput=False)
    # split-identity lhsT for the bias matmul: [p, i*128+m] = (m == p+64i)
    id8 = nc.declare_dram_parameter("id8", [64, 256], FP8, isOutput=False)
    # bias rhs tiles: [uid, p, i*mw + q] = MBIAS*(1-mask[p+64i, q])
    b8 = nc.declare_dram_parameter("b8", [max(n_uniq, 1), 64, 2 * mw], FP8,
                                   isOutput=False)
    # bf16 variants for chunk-0 (a PSUM accumulation group cannot mix
    # matmul dtypes/modes): full identity + bias [uid, p(k), q]
    idb = nc.declare_dram_parameter("idb", [128, 128], BF16, isOutput=False)
    bb = nc.declare_dram_parameter("bb", [max(n_uniq, 1), 128, mw], BF16,
                                   isOutput=False)
    ot = nc.declare_dram_parameter("ot", [n_groups, NQ, 128, 8 * VW], F32,
                                   isOutput=True)

    engplan, _ = _plan_engines(S, n_groups, plans)
    sch_b = SCH_B - CSH * SCH_A

    with tile.TileContext(nc) as tc, ExitStack() as ctx:
        qpool = ctx.enter_context(tc.tile_pool(name="qpool", bufs=2))
        kpool = ctx.enter_context(tc.tile_pool(name="kpool", bufs=2))
        vpool = ctx.enter_context(tc.tile_pool(name="vpool", bufs=4))
        mpool = ctx.enter_context(tc.tile_pool(name="mpool", bufs=1))
        ppool = ctx.enter_context(tc.tile_pool(name="ppool", bufs=p_bufs))
        obuf = ctx.enter_context(tc.tile_pool(name="obuf", bufs=osb_bufs))
        spool = ctx.enter_context(tc.tile_pool(name="spool", bufs=s_bufs,
                                               space="PSUM"))
        opool = ctx.enter_context(tc.tile_pool(name="opool", bufs=2,
                                               space="PSUM"))

        warm = mpool.tile([128, 8], F32)
        idt = mpool.tile([64, 256], FP8)
        bt = mpool.tile([64, max(n_uniq, 1) * 2 * mw], FP8)
        idtb = mpool.tile([128, 128], BF16)
        btb = mpool.tile([128, max(n_uniq, 1) * mw], BF16)
        cbias = mpool.tile([128, 1], F32)

        def exp_emit(eng, dst, src):
            if eng == "a" or not USE_SCHRAUD:
                nc.scalar.activation(dst, src, EXP, bias=cbias[:])
            elif eng == "v":
                nc.vector.tensor_scalar(dst.bitcast(I16), src, SCH_A, sch_b,
                                        MULT, ADD)
            else:
                nc.gpsimd.tensor_scalar(dst.bitcast(I16), src, SCH_A, sch_b,
                                        MULT, ADD)

        giter = list(range(n_groups))
        # flatten (group, chunk, block) into one stream so the P@V retire
        # lag (LA) pipelines ACROSS chunk and group boundaries. Chunk-0
        # blocks (exp pinned to ACT for precision) are WOVEN between
        # chunk-1 blocks so the 3-deep PSUM score ring always holds work
        # for both exp engines; retirement is chunk-grouped (qc1 fully
        # retires before qc0 starts) so only one P@V output chunk is open.
        flat = []
        rrank = []  # (group, chunk_rank, bi) for retire ordering
        crank = {1: 0, 0: 1, 2: 2, 3: 3}
        for (gi, qc, bi, blk, nb) in _flat_order(plans, n_groups):
            flat.append((gi, giter[gi], qc, bi, blk, nb))
            rrank.append((gi, crank.get(qc, qc), bi))
        N = len(flat)
        LA = la
        retire_list = sorted(range(N), key=lambda j: rrank[j])
        gtiles = {}
        cstate = {}
        staged = []

        def emit_group_loads(gi, g):
            ktile = kpool.tile([128, QBLK], BF16, tag="kt")
            qtile = qpool.tile([128, QBLK], BF16, tag="qt")
            k8tile = kpool.tile([64, 2 * NK * 2 * KBLK], FP8, tag="k8t")
            q8tile = qpool.tile([64, 2 * (S - QBLK)], FP8, tag="q8t")
            vtiles = [vpool.tile([128, NK * VW], BF16, tag=f"vt{h}",
                                 name=f"vt{h}") for h in range(2)]
            # first-needed-first for the woven order: chunk 1 (fp8) opens
            # the group, chunk-0 bf16 blocks follow two emissions later
            nq4 = 4 * VW
            nk8 = 4 * 2 * KBLK
            HK8 = NK * 2 * KBLK
            HQ8 = S - QBLK
            if gi == 0:
                nc.sync.dma_start(k8tile[:, 0:2 * nk8], k8[g, :, 0:2 * nk8])
                nc.scalar.dma_start(q8tile[:, 0:QBLK], q8[g, :, 0:QBLK])
                # ACT exp-table load + const tiles overlap the initial DMAs
                nc.vector.memset(warm[:], 0.0)
                nc.vector.memset(cbias[:], -CSH)
                nc.scalar.activation(warm[:], warm[:], EXP)
                nc.gpsimd.dma_start(idt[:], id8[:])
                nc.gpsimd.dma_start(idtb[:], idb[:])
                for u in range(n_uniq):
                    nc.gpsimd.dma_start(bt[:, u * 2 * mw:(u + 1) * 2 * mw],
                                        b8[u])
                    nc.gpsimd.dma_start(btb[:, u * mw:(u + 1) * mw], bb[u])
            else:
                nc.sync.dma_start(k8tile[:, 0:2 * nk8], k8[g, :, 0:2 * nk8])
                nc.sync.dma_start(q8tile[:, 0:QBLK], q8[g, :, 0:QBLK])
            nc.sync.dma_start(k8tile[:, HK8:HK8 + 2 * nk8],
                              k8[g, :, HK8:HK8 + 2 * nk8])
            nc.sync.dma_start(q8tile[:, HQ8:HQ8 + QBLK],
                              q8[g, :, HQ8:HQ8 + QBLK])
            nc.sync.dma_start(ktile[:], ktb[g])
            nc.sync.dma_start(qtile[:], qtb[g])
            for h in range(2):
                nc.sync.dma_start(vtiles[h][:, 0:2 * nq4],
                                  vv[2 * g + h, :, 0:2 * nq4])
            for qc in range(2, NQ):
                for h in range(2):
                    nc.sync.dma_start(
                        q8tile[:, h * HQ8 + (qc - 1) * QBLK:
                               h * HQ8 + qc * QBLK],
                        q8[g, :, h * HQ8 + (qc - 1) * QBLK:h * HQ8 + qc * QBLK])
                for h in range(2):
                    nc.sync.dma_start(
                        k8tile[:, h * HK8 + qc * nk8:
                               h * HK8 + (qc + 1) * nk8],
                        k8[g, :, h * HK8 + qc * nk8:h * HK8 + (qc + 1) * nk8])
                for h in range(2):
                    nc.sync.dma_start(
                        vtiles[h][:, qc * nq4:(qc + 1) * nq4],
                        vv[2 * g + h, :, qc * nq4:(qc + 1) * nq4])
            gtiles[gi] = (ktile, qtile, k8tile, q8tile, vtiles)

        # retire lag per block: the first blocks of each chunk (in retire
        # order) wait extra fills so the previous chunk's PSUM-bank copy
        # (WAR on o_bufs=1) completes before their start=True matmul needs
        # the bank
        xlag = int(os.environ.get("K_XLAG", "2"))
        lag = [LA] * N
        seen = {}
        for j in retire_list:
            gi, g, qc, bi, blk, nb = flat[j]
            k = seen.get((gi, qc), 0)
            if k < 2:
                lag[j] = LA + xlag
            seen[(gi, qc)] = k + 1
        rptr = 0
        for idx in range(N + N):
            if rptr >= N and idx >= N:
                break
            if idx < N:
                gi, g, qc, bi, blk, nb = flat[idx]
                if gi not in gtiles:
                    emit_group_loads(gi, g)
                # prefetch the next group's loads while this group still has
                # a full chunk of compute in flight
                if qc == NQ - 1 and bi == 0 and gi + 1 < len(giter) \
                        and gi + 1 not in gtiles:
                    emit_group_loads(gi + 1, giter[gi + 1])
                ktile, qtile, k8tile, q8tile, _ = gtiles[gi]
                kk, c0, c1, m0, m1, uid = blk
                W = c1 - c0
                diag = uid >= 0 and not NOBIAS
                s_ps = spool.tile([128, 2 * QBLK], F32, tag="s")
                p_t = ppool.tile([128, 2 * QBLK], BF16, tag="p")
                use8 = USE_FP8 and qc >= 1
                for h in range(2):
                    dst = s_ps[:, h * QBLK + c0:h * QBLK + c1]
                    if use8:
                        hk0 = h * (NK * 2 * KBLK)
                        lhsT = k8tile[:, hk0 + kk * 2 * KBLK:
                                      hk0 + (kk + 1) * 2 * KBLK] \
                            .rearrange("p (i m) -> p i m", i=2)
                        # moving free dim is 2*W in DoubleRow mode; the HW
                        # cap is 512, so emit <=256-col pieces
                        for w0 in range(0, W, 256):
                            w1 = min(w0 + 256, W)
                            rq0 = h * (S - QBLK) + (qc - 1) * QBLK + c0 + w0
                            rhs = q8tile[:, rq0:rq0 + w1 - w0] \
                                .unsqueeze(1).broadcast_to((64, 2, w1 - w0))
                            nc.tensor.matmul(
                                s_ps[:, h * QBLK + c0 + w0:h * QBLK + c0 + w1],
                                lhsT=lhsT, rhs=rhs,
                                start=w0 == 0,
                                stop=(not diag) and w1 == W,
                                perf_mode=DR)
                    else:
                        nc.tensor.matmul(
                            dst,
                            lhsT=ktile[64 * h:64 * h + 64,
                                       kk * KBLK:(kk + 1) * KBLK],
                            rhs=qtile[64 * h:64 * h + 64, c0:c0 + W],
                            start=True, stop=not diag)
                    if diag:
                        # mask-bias matmul: accumulate MBIAS onto masked
                        # entries of the dirty strip (closes the PSUM group);
                        # must match the S matmul's dtype/mode
                        if use8:
                            lhsT = idt[:].rearrange("p (i m) -> p i m", i=2)
                            rhs = bt[:, uid * 2 * mw:(uid + 1) * 2 * mw] \
                                .rearrange("p (i q) -> p i q", i=2)
                            rhs = rhs[:, :, 0:m1 - m0]
                            nc.tensor.matmul(
                                s_ps[:, h * QBLK + m0:h * QBLK + m1],
                                lhsT=lhsT, rhs=rhs,
                                start=False, stop=True, perf_mode=DR)
                        else:
                            nc.tensor.matmul(
                                s_ps[:, h * QBLK + m0:h * QBLK + m1],
                                lhsT=idtb[:],
                                rhs=btb[:, uid * mw:uid * mw + m1 - m0],
                                start=False, stop=True)
                eng = engplan[(gi, qc, kk)]
                if W == QBLK:
                    if idx == N - 1:
                        # drain tail: split the final exp across engines
                        e0 = os.environ.get("K_DR", "av")
                        nsp = len(e0)
                        step = (2 * QBLK) // nsp
                        for si in range(nsp):
                            exp_emit(e0[si],
                                     p_t[:, si * step:(si + 1) * step],
                                     s_ps[:, si * step:(si + 1) * step])
                    else:
                        exp_emit(eng, p_t[:, 0:2 * QBLK], s_ps[:, 0:2 * QBLK])
                else:
                    # one strided instruction covers both heads' [c0, c1)
                    sv = s_ps[:].rearrange("p (a q) -> p a q", a=2)
                    pv = p_t[:].rearrange("p (a q) -> p a q", a=2)
                    exp_emit(eng, pv[:, :, c0:c1], sv[:, :, c0:c1])
                staged.append((flat[idx], p_t))
            while rptr < N and idx - retire_list[rptr] >= lag[retire_list[rptr]]:
                r = retire_list[rptr]
                rptr += 1
                (gi, g, qc, bi, blk, nb), p_t = staged[r]
                kk, c0, c1, m0, m1, uid = blk
                W = c1 - c0
                vtiles = gtiles[gi][4]
                st = cstate.get((gi, qc))
                if st is None:
                    # one PSUM tile for both heads: h0 at cols [0, 260) in
                    # zero-region 0, h1 at [512, 772) in region 1 -> a single
                    # strided copy moves both heads
                    o_ps = opool.tile([128, 1024], F32, tag="o",
                                      name="o_ps", bufs=o_bufs)
                    n_pv = sum((b[2] - b[1]) // KBLK for b in plans[qc])
                    st = {"o": o_ps, "n": n_pv, "c": [0, 0]}
                    cstate[(gi, qc)] = st
                o_ps = st["o"]
                for h in range(2):
                    for jj in range(c0 // KBLK, (c1 + KBLK - 1) // KBLK):
                        je = min((jj + 1) * KBLK, c1)
                        M = je - jj * KBLK
                        st["c"][h] += 1
                        nc.tensor.matmul(
                            o_ps[0:M, h * QBLK + jj * VW:
                                 h * QBLK + (jj + 1) * VW],
                            lhsT=p_t[:, h * QBLK + jj * KBLK:h * QBLK + je],
                            rhs=vtiles[h][:, kk * VW:(kk + 1) * VW],
                            start=st["c"][h] == 1,
                            stop=st["c"][h] == st["n"])
                if bi == nb - 1:
                    del cstate[(gi, qc)]
                    is_final = rptr == N
                    osb = obuf.tile([128, 8 * VW], F32, tag="osb")
                    eng = engplan[("copy", gi, qc)]
                    src = o_ps[:].rearrange("p (a q) -> p a q", a=2)
                    dsl = osb[:].rearrange("p (a q) -> p a q", a=2)
                    if is_final:
                        # drain: split the last copy across both engines and
                        # fire each half's store as soon as its copy lands
                        nc.scalar.copy(dsl[:, 0, 0:4 * VW], src[:, 0, 0:4 * VW])
                        nc.sync.dma_start(ot[g, qc][:, 0:4 * VW],
                                          osb[:, 0:4 * VW])
                        nc.vector.tensor_copy(dsl[:, 1, 0:4 * VW],
                                              src[:, 1, 0:4 * VW])
                    elif eng == "a":
                        nc.scalar.copy(dsl[:, :, 0:4 * VW],
                                       src[:, :, 0:4 * VW])
                    else:
                        nc.vector.tensor_copy(dsl[:, :, 0:4 * VW],
                                              src[:, :, 0:4 * VW])
                    dst = ot[g, qc]
                    if is_final:
                        # h0 store already fired right after its copy
                        nc.scalar.dma_start(dst[:, 4 * VW:], osb[:, 4 * VW:])
                    else:
                        # SWDGE queue: an out-store waiting on its copy must
                        # not block the SP queue head
                        nc.gpsimd.dma_start(dst, osb[:])
    nc.finalize()
    return nc


def _make_in_maps(q4, k4, v4, uniq, n_groups, per_core):
    import ml_dtypes
    B, S = q4.shape[0], q4.shape[1]
    NK = S // KBLK
    n_uniq = len(uniq)
    mw = uniq[0].shape[2] if uniq else 1
    FP8NP = ml_dtypes.float8_e4m3

    # split-identity: id8[p, i*128 + m] = 1.0 if m == p + 64*i
    ident = np.zeros((64, 256), np.float32)
    for p in range(64):
        ident[p, p] = 1.0
        ident[p, 128 + 64 + p] = 1.0

    in_maps = []
    for c in range(N_CORES):
        qtb = np.empty((n_groups, 128, QBLK), np.float32)
        ktb = np.empty((n_groups, 128, QBLK), np.float32)
        k8a = np.empty((n_groups, 64, 2 * NK * 2 * KBLK), FP8NP)
        q8a = np.empty((n_groups, 64, 2 * (S - QBLK)), FP8NP)
        vvv = np.empty((per_core, 128, NK * VW), np.float32)
        HK8 = NK * 2 * KBLK
        HQ8 = S - QBLK
        bs = []
        for lp in range(per_core):
            gp = c * per_core + lp
            b, h = divmod(gp, H)
            bs.append(b)
            g, half = divmod(lp, 2)
            sl = slice(64 * half, 64 * half + 64)
            Q = q4[b, :, h, :].T  # [dh, S]
            K = k4[b, :, h, :].T
            qtb[g, sl] = Q[:, :QBLK]
            ktb[g, sl] = K[:, :QBLK]
            q8a[g, :, half * HQ8:(half + 1) * HQ8] = Q[:, QBLK:].astype(FP8NP)
            khi = K.astype(FP8NP)
            klo = (K - khi.astype(np.float32)).astype(FP8NP)
            # [dh, kk*256 + i*128 + m]
            k8v = np.empty((64, NK, 2, KBLK), FP8NP)
            k8v[:, :, 0, :] = khi.reshape(64, NK, KBLK)
            k8v[:, :, 1, :] = klo.reshape(64, NK, KBLK)
            k8a[g, :, half * HK8:(half + 1) * HK8] = k8v.reshape(64, HK8)
            vt = np.ones((128, NK, VW), np.float32)
            vt[:, :, :DH] = v4[b, :, h, :].reshape(NK, KBLK, DH).transpose(1, 0, 2)
            vvv[lp] = vt.reshape(128, NK * VW)
        if n_uniq:
            assert len(set(bs)) == 1, "bias tiles assume one batch per core"
            b8arr = np.zeros((n_uniq, 64, 2 * mw), np.float32)
            bbarr = np.zeros((n_uniq, 128, mw), np.float32)
            for u in range(n_uniq):
                cont = uniq[u][bs[0]]  # [128 k, mw] 1=allowed
                bias = MBIAS * (1.0 - cont)  # [128, mw]
                b8arr[u, :, 0:mw] = bias[0:64]
                b8arr[u, :, mw:2 * mw] = bias[64:128]
                bbarr[u] = bias
        else:
            b8arr = np.zeros((1, 64, 2), np.float32)
            bbarr = np.zeros((1, 128, 1), np.float32)
        in_maps.append({
            "qtb": qtb.astype(ml_dtypes.bfloat16),
            "ktb": ktb.astype(ml_dtypes.bfloat16),
            "k8": k8a,
            "q8": q8a,
            "vv": vvv.astype(ml_dtypes.bfloat16),
            "id8": ident.astype(FP8NP),
            "b8": b8arr.astype(FP8NP),
            "idb": np.eye(128, dtype=np.float32).astype(ml_dtypes.bfloat16),
            "bb": bbarr.astype(ml_dtypes.bfloat16),
        })
    return in_maps


def _assemble(results, B, S, per_core):
    D = H * DH
    out = np.empty((B, S, D), np.float32)
    for c in range(N_CORES):
        otc = results[c]["ot"]
        for lp in range(per_core):
            gp = c * per_core + lp
            b, h = divmod(gp, H)
            g, half = divmod(lp, 2)
            # otc: [n_groups, NQ, 128, 2 (head), 4 (sub), VW]
            o = otc[g].reshape(S // QBLK, 128, 2, 4, VW)[:, :, half]
            o = o.transpose(0, 2, 1, 3).reshape(S, VW).astype(np.float64)
            l = o[:, DH]
            l = np.where(l == 0.0, 1.0, l)
            out[b, :, h * DH:(h + 1) * DH] = \
                (o[:, :DH] / l[:, None]).astype(np.float32)
    return out


def kernel(queries, keys, values, mask):
    B, S, D = queries.shape
    assert D == H * DH
    q4 = (np.ascontiguousarray(queries, dtype=np.float32) * 0.125) \
        .reshape(B, S, H, DH)
    k4 = np.ascontiguousarray(keys, dtype=np.float32).reshape(B, S, H, DH)
    v4 = np.ascontiguousarray(values, dtype=np.float32).reshape(B, S, H, DH)
    maskb = np.asarray(mask).astype(bool)

    plans, uniq = _plan_blocks(maskb)
    per_core = (B * H) // N_CORES
    n_groups = per_core // 2

    mw = uniq[0].shape[2] if uniq else 1
    nc = _build(S, n_groups, per_core, plans, len(uniq), mw=mw)
    in_maps = _make_in_maps(q4, k4, v4, uniq, n_groups, per_core)
    try:
        res = run_bass_kernel_spmd(nc, in_maps, core_ids=list(range(N_CORES)))
    except ModuleNotFoundError:
        os.environ["BASS_NEVER_TRACE"] = "1"
        res = run_bass_kernel_spmd(nc, in_maps, core_ids=list(range(N_CORES)))
    global LAST_RESULTS
    LAST_RESULTS = res
    return _assemble(res.results, B, S, per_core)
